# revision 7
# baseline (speedup 1.0000x reference)
"""BatchHardTripletLoss kernel for 8 Trainium2 NeuronCores.

Math (matches the jax reference):
  dist2[i,j] = |e1_i|^2 + |e2_j|^2 - 2 e1.e2 + 2*eps*(s1_i - s2_j) + D*eps^2
             = a[i] + v[i,j],   v[i,j] = b[j] - 2*G[i,j]
  pos_max[i] = sqrt(max_{j in pos} dist2), neg_min[i] = sqrt(min_{j in neg})
  loss = mean over pos anchors of relu(pos_max - neg_min + margin)

Only rows with target[i]==1 contribute, so the device computes pos anchors
only (~k/8 rows per core).  The k % 1024 leftover anchors and the pos
columns beyond the 4*PW device budget are done exactly on the host (small
numpy job).

Device strategy (data parallel over pos-anchor rows, 8 cores):
  - Host lays candidate columns out as four 2048-col pairs, each pair =
    PW pos cols then 2048-PW neg cols, so every pair feeds BOTH consumer
    engines at once (PW tuned so ScalarE and VectorE finish together).
    emb2 goes up transposed [D, cols] bf16; emb1 is scaled by -2; the
    bias b[j] enters PSUM via K=4 row-packed tail matmuls (dims 126/127
    + bf16 hi/lo split of b) next to the K=126 mains.  Neg cols are
    padded with duplicates.
  - PSUM is one [128, 4096] f32 tile = ring of two 2048-col halves.  Per
    (i-tile of 128 anchors, pair) TensorE fills one half; pairs alternate
    halves; the Tile framework's range-level dependencies give
    region-granular pipelining.  In steady state all three engines are
    >95% busy and the PE never idles, which keeps the HAM activity
    clock-gate at 2.4 GHz; ~30 warm-up matmuls on a tiny first-loaded
    tensor pre-fill one 4096-cycle activity window while the bulk DMAs
    stream in (split across the Sync and Scalar HWDGE queues).
  - Consumers per pair, concurrent:
      neg cols -> VectorE exact tensor_reduce(min) of v.
      pos cols -> ScalarE activation(Exp, scale=T, bias=per-row AP,
        accum_out): the sum of exp(T*(dist2 - K_i)) is a log-sum-exp max
        finished on the host with one ln(); the per-row offset
        K_i = a_i + c1 + c2*sqrt(a_i) + U keeps every row inside the f32
        exp window.  Rows outside the window (none on real data) are
        detected on the host and recomputed exactly.
  - Host: ln/sqrt/relu/mean in f64 (O(k) work) + the small exact slab.
"""

import sys

for _p in ("/opt/trn_rl_repo",):
    if _p not in sys.path:
        sys.path.insert(0, _p)

import numpy as np
import ml_dtypes

EPS = 1e-6
MARGIN = 0.2
B = 8192
D = 128
NCORES = 8
PAIRW = 2048         # column pair = 4 PSUM banks
NPAIR = B // PAIRW
WARMUP_MM = 30       # N=128 dummy matmuls to warm the HAM clock gate

# log-sum-exp calibration (fit to randn(B,128) stats; host detector +
# exact-row fallback covers anything outside the window)
T = 1.0
BIASP = lambda sa: -(131.26 + 22.0 + 10.20 * sa) * T   # exp arg = T*v + bias

_programs = {}
LAST_RESULTS = None   # BassKernelResults of the most recent run (for profiling)


def _pos_width(k):
    """Pos cols per pair; 928 balances ScalarE vs VectorE per-pair time."""
    return min(928, (k // NPAIR) // 16 * 16)


def _build_program(pw, kp):
    import concourse.bacc as bacc
    import concourse.tile as tile
    from concourse import mybir

    f32 = mybir.dt.float32
    bf16 = mybir.dt.bfloat16
    AOT = mybir.AluOpType
    AFT = mybir.ActivationFunctionType

    SH = kp // NCORES
    NIT = SH // 128
    nslot = 2 * NPAIR

    nc = bacc.Bacc(None)
    wrm = nc.declare_dram_parameter("wrm", [D, 128], bf16, isOutput=False)
    e1t = nc.declare_dram_parameter("e1t", [D, SH], bf16, isOutput=False)
    e2t = nc.declare_dram_parameter("e2t", [D, B], bf16, isOutput=False)
    tailw = nc.declare_dram_parameter("tailw", [16, SH], bf16, isOutput=False)
    trhs = nc.declare_dram_parameter("trhs", [16, B], bf16, isOutput=False)
    biasp = nc.declare_dram_parameter("biasp", [128, NIT], f32, isOutput=False)
    outp = nc.declare_dram_parameter("out", [128, NIT * nslot], f32, isOutput=True)

    with tile.TileContext(nc) as tc:
        with (
            tc.tile_pool(name="const", bufs=1) as cpool,
            tc.tile_pool(name="e2p", bufs=2 * NPAIR) as e2pool,
            tc.tile_pool(name="ps", bufs=1, space="PSUM") as pspool,
        ):
            # sync HWDGE queue: warmup tensor, e1, first chunks, trhs, rest
            wrsb = cpool.tile([D, 128], bf16, tag="wrsb")
            nc.sync.dma_start(wrsb[:], wrm[:])
            e1sb = cpool.tile([D, SH], bf16, tag="e1sb")
            nc.sync.dma_start(e1sb[:], e1t[:])
            e2sb = []
            def load_chunk(g):
                e2c = e2pool.tile([D, 1024], bf16, tag=f"e2c{g}")
                nc.sync.dma_start(e2c[:], e2t[:, g * 1024:(g + 1) * 1024])
                e2sb.append(e2c)
            load_chunk(0)
            load_chunk(1)
            trsb = cpool.tile([128, B], bf16, tag="trsb")
            for s in range(4):
                nc.sync.dma_start(trsb[32 * s:32 * s + 4, :], trhs[4 * s:4 * s + 4, :])
            for g in range(2, 2 * NPAIR):
                load_chunk(g)

            # scalar HWDGE queue: the small fixed inputs (parallel issue)
            bpsb = cpool.tile([128, NIT], f32, tag="bpsb")
            nc.scalar.dma_start(bpsb[:], biasp[:])
            twsb = cpool.tile([128, SH], bf16, tag="twsb")
            for s in range(4):
                nc.scalar.dma_start(
                    twsb[32 * s:32 * s + 4, :], tailw[4 * s:4 * s + 4, :]
                )

            outsb = cpool.tile([128, NIT * nslot], f32, tag="outsb")
            scr = cpool.tile([128, PAIRW], f32, tag="scr")

            # kick the Exp table load early so it overlaps input DMA
            nc.scalar.activation(scr[:, 0:1], bpsb[:, 0:1], AFT.Exp)

            ps = pspool.tile([128, 4096], f32, tag="ps")

            # warm the PE activity window with throwaway matmuls
            for _ in range(WARMUP_MM):
                nc.tensor.matmul(
                    ps[:, 0:128], wrsb[0:126, 0:128], wrsb[0:126, 0:128],
                    start=True, stop=True,
                )

            slot = 0
            for it in range(NIT):
                icols = slice(it * 128, (it + 1) * 128)
                w = e1sb[0:126, icols]
                for pi in range(NPAIR):
                    roff = (pi % 2) * PAIRW      # PSUM half for this pair
                    # K=126 mains (embedding dims 0..125)
                    for gi in range(2):
                        g = 2 * pi + gi
                        for s in range(2):
                            c0 = roff + gi * 1024 + s * 512
                            nc.tensor.matmul(
                                ps[:, c0:c0 + 512],
                                w,
                                e2sb[g][0:126, s * 512:(s + 1) * 512],
                                start=True,
                                stop=False,
                            )
                    # K=4 tails (dims 126,127 + bias hi/lo), 4-way row-packed
                    # so the four 512-wide sub-tiles run concurrently
                    for si in range(4):
                        c0 = roff + si * 512
                        j0 = pi * PAIRW + si * 512
                        nc.tensor.matmul(
                            ps[:, c0:c0 + 512],
                            twsb[32 * si:32 * si + 4, icols],
                            trsb[32 * si:32 * si + 4, j0:j0 + 512],
                            start=False,
                            stop=True,
                            tile_position=(32 * si, 0),
                        )
                    # concurrent consumers: pos cols -> ScalarE, neg -> VectorE
                    nc.scalar.activation(
                        scr[:, 0:pw], ps[:, roff:roff + pw], AFT.Exp,
                        bias=bpsb[:, it:it + 1],
                        scale=T,
                        accum_out=outsb[:, slot:slot + 1],
                    )
                    nc.vector.tensor_reduce(
                        outsb[:, slot + 1:slot + 2],
                        ps[:, roff + pw:roff + PAIRW],
                        axis=mybir.AxisListType.X, op=AOT.min,
                    )
                    slot += 2
                # drain this i-tile's results while compute continues
                nc.sync.dma_start(
                    outp[:, it * nslot:(it + 1) * nslot],
                    outsb[:, it * nslot:(it + 1) * nslot],
                )
    nc.compile()
    return nc


def _host_prep(emb1, emb2, target, kp):
    tpos = target == 1
    k = int(tpos.sum())
    pos_idx = np.nonzero(tpos)[0]
    perm = np.concatenate([pos_idx, np.nonzero(~tpos)[0]])
    e2s = emb2[perm]
    e2d = e2s.astype(np.float64)
    b = (e2d * e2d).sum(1) - (2.0 * EPS) * e2d.sum(1)

    # device column order: per pair, pw pos cols then 2048-pw neg cols
    pw = _pos_width(k)
    colperm = np.empty(B, dtype=np.int64)
    pneg = k
    off = 0
    for p in range(NPAIR):
        colperm[off:off + pw] = np.arange(p * pw, (p + 1) * pw)
        n = min(PAIRW - pw, B - pneg)
        colperm[off + pw:off + pw + n] = np.arange(pneg, pneg + n)
        if n < PAIRW - pw:  # pad with duplicates of the first neg column
            colperm[off + pw + n:off + PAIRW] = k
        pneg += n
        off += PAIRW

    e1dev = emb1[pos_idx[:kp]]
    e1d = e1dev.astype(np.float64)
    a = (e1d * e1d).sum(1) + (2.0 * EPS) * e1d.sum(1) + D * EPS * EPS
    e1tb = np.ascontiguousarray((-2.0 * e1dev).T.astype(ml_dtypes.bfloat16))
    e2dev = e2s[colperm]
    bdev = b[colperm]
    e2tb = np.ascontiguousarray(e2dev.T.astype(ml_dtypes.bfloat16))
    bhi = bdev.astype(np.float32).astype(ml_dtypes.bfloat16)
    blo = (bdev.astype(np.float32) - bhi.astype(np.float32)).astype(ml_dtypes.bfloat16)
    # K=4 tail operands; on device row 4s+r lands at partition 32s+r so the
    # four 512-wide sub-tiles of a pair can row-pack on the PE array.
    tailw = np.zeros((16, kp), dtype=ml_dtypes.bfloat16)
    trhs = np.zeros((16, B), dtype=ml_dtypes.bfloat16)
    one = np.ones(B, dtype=ml_dtypes.bfloat16)
    for s in range(4):
        tailw[4 * s + 0] = e1tb[126]
        tailw[4 * s + 1] = e1tb[127]
        tailw[4 * s + 2] = one[:kp]
        tailw[4 * s + 3] = one[:kp]
        trhs[4 * s + 0] = e2tb[126]
        trhs[4 * s + 1] = e2tb[127]
        trhs[4 * s + 2] = bhi
        trhs[4 * s + 3] = blo
    bp = BIASP(np.sqrt(a)).astype(np.float32)   # exp arg = T*v + bp
    return k, pw, a, b, e2d, pos_idx, e1tb, e2tb, tailw, trhs, bp


def _exact_rows(e1rows, e2d, b, k):
    """Exact f64 pos_max2/neg_min2 for a handful of anchor rows."""
    e1d = e1rows.astype(np.float64)
    av = (e1d * e1d).sum(1) + (2.0 * EPS) * e1d.sum(1) + D * EPS * EPS
    d2 = av[:, None] + b[None, :] - 2.0 * (e1d @ e2d.T)
    return d2[:, :k].max(1), d2[:, k:].min(1)


def _numpy_fallback(emb1, emb2, target):
    e1 = emb1.astype(np.float64)
    e2 = emb2.astype(np.float64)
    sq = (
        (e1 * e1).sum(1)[:, None]
        + (e2 * e2).sum(1)[None, :]
        - 2.0 * (e1 @ e2.T)
        + 2.0 * EPS * (e1.sum(1)[:, None] - e2.sum(1)[None, :])
        + D * EPS * EPS
    )
    dist = np.sqrt(np.clip(sq, 0.0, None))
    pos = target == 1
    neg = target == 0
    pos_max = np.where(pos[None, :], dist, -np.inf).max(1)
    neg_min = np.where(neg[None, :], dist, np.inf).min(1)
    per = np.maximum(pos_max - neg_min + MARGIN, 0.0)
    w = pos.astype(np.float64)
    return np.float32((per * w).sum() / w.sum())


def kernel(emb1, emb2, target):
    global LAST_RESULTS
    emb1 = np.asarray(emb1, dtype=np.float32)
    emb2 = np.asarray(emb2, dtype=np.float32)
    target = np.asarray(target)
    assert emb1.shape == (B, D) and emb2.shape == (B, D)

    k = int((target == 1).sum())
    kp = (k // (NCORES * 128)) * (NCORES * 128)
    if kp == 0 or k >= B - 1 or _pos_width(k) <= 0:
        return _numpy_fallback(emb1, emb2, target)

    k, pw, a, b, e2d, pos_idx, e1tb, e2tb, tailw, trhs, bp = _host_prep(
        emb1, emb2, target, kp
    )

    nc = _programs.get((pw, kp))
    if nc is None:
        nc = _build_program(pw, kp)
        _programs[(pw, kp)] = nc

    from concourse.bass_utils import run_bass_kernel_spmd

    SH = kp // NCORES
    NIT = SH // 128
    in_maps = [
        {
            "wrm": np.ascontiguousarray(e1tb[:, 0:128]),
            "e1t": np.ascontiguousarray(e1tb[:, c * SH:(c + 1) * SH]),
            "e2t": e2tb,
            "tailw": np.ascontiguousarray(tailw[:, c * SH:(c + 1) * SH]),
            "trhs": trhs,
            "biasp": np.ascontiguousarray(
                bp[c * SH:(c + 1) * SH].reshape(NIT, 128).T
            ),
        }
        for c in range(NCORES)
    ]
    res = run_bass_kernel_spmd(nc, in_maps, core_ids=list(range(NCORES)))
    LAST_RESULTS = res

    # ---- host reconstruction (all f64) ----
    nslot = 2 * NPAIR
    Sp = np.zeros(kp)
    vmin = np.full(kp, np.inf)
    for c in range(NCORES):
        out = np.asarray(res.results[c]["out"]).astype(np.float64)  # [128, NIT*nslot]
        for it in range(NIT):
            rows = slice(c * SH + it * 128, c * SH + (it + 1) * 128)
            blk = out[:, it * nslot:(it + 1) * nslot]
            Sp[rows] += blk[:, 0::2].sum(1)
            vmin[rows] = np.minimum(vmin[rows], blk[:, 1::2].min(1))

    bp64 = bp.astype(np.float64)
    with np.errstate(divide="ignore", invalid="ignore"):
        pm2 = a + (np.log(Sp) - bp64) / T
    nm2 = a + vmin

    # exact host slab: pos cols [NPAIR*pw, k)
    if k > NPAIR * pw:
        e1d = emb1[pos_idx[:kp]].astype(np.float64)
        d2s = (
            a[:, None]
            + b[None, NPAIR * pw:k]
            - 2.0 * (e1d @ e2d[NPAIR * pw:k].T)
        )
        pm2 = np.maximum(pm2, d2s.max(1))

    # detector: rows where the LSE left the reliable window -> exact redo
    bad = (~np.isfinite(Sp)) | (Sp <= 0) | (np.log(np.maximum(Sp, 1e-300)) < -60.0)
    bad |= ~np.isfinite(pm2) | ~np.isfinite(nm2)
    if bad.any():
        idx = np.nonzero(bad)[0]
        pmx, nmx = _exact_rows(emb1[pos_idx[idx]], e2d, b, k)
        pm2[idx] = pmx
        nm2[idx] = nmx

    per = np.maximum(
        np.sqrt(np.clip(pm2, 0.0, None)) - np.sqrt(np.clip(nm2, 0.0, None)) + MARGIN,
        0.0,
    )
    total = per.sum()

    if k > kp:  # leftover pos anchors, exact on host
        pmx, nmx = _exact_rows(emb1[pos_idx[kp:k]], e2d, b, k)
        total += np.maximum(
            np.sqrt(np.clip(pmx, 0.0, None)) - np.sqrt(np.clip(nmx, 0.0, None)) + MARGIN,
            0.0,
        ).sum()

    return np.float32(total / k)


# revision 8
# speedup vs baseline: 1.1294x; 1.1294x over previous
"""BatchHardTripletLoss kernel for 8 Trainium2 NeuronCores.

Math (matches the jax reference):
  dist2[i,j] = |e1_i|^2 + |e2_j|^2 - 2 e1.e2 + 2*eps*(s1_i - s2_j) + D*eps^2
             = a[i] + v[i,j],   v[i,j] = b[j] - 2*G[i,j]
  pos_max[i] = sqrt(max_{j in pos} dist2), neg_min[i] = sqrt(min_{j in neg})
  loss = mean over pos anchors of relu(pos_max - neg_min + margin)

Only rows with target[i]==1 contribute, so the device computes pos anchors
only (~k/8 rows per core).  The k % 1024 leftover anchors and the pos
columns past the last full 1024-chunk are done exactly on the host (tiny
numpy job).

Device strategy (data parallel over pos-anchor rows, 8 cores):
  - Host INTERLEAVES pos/neg candidate columns in 1024-col chunks
    (P N P N ...), so every 2048-col chunk pair feeds BOTH consumer
    engines at once (chunk boundaries stay PSUM-bank aligned -- a
    mid-bank split makes ScalarE and VectorE contend on one bank).
    emb2 goes up transposed [D, cols] bf16; emb1 is scaled by -2; the
    bias b[j] enters PSUM via K=4 row-packed tail matmuls (dims 126/127
    + bf16 hi/lo split of b) next to the K=126 mains.  Neg chunks are
    padded to 1024 with duplicate neg columns.
  - PSUM is one [128, 4096] f32 tile = ring of two 2048-col halves.  Per
    (i-tile of 128 anchors, chunk pair) TensorE fills one half; pairs
    alternate halves; the Tile framework's range-level dependencies give
    region-granular pipelining.  In steady state all three engines are
    >90% busy and the PE never idles, which keeps the HAM activity
    clock-gate at 2.4 GHz (the first couple of cold pairs warm it up).
    Input DMAs are split across the Sync (e1, e2 chunks in use order)
    and Scalar (bias + combined tail strips) HWDGE queues so the first
    pair can start ~1 DMA-issue after the framework preamble.
  - Consumers run concurrently per pair:
      neg chunk -> VectorE exact tensor_reduce(min) of v.
      pos chunk -> ScalarE activation(Exp, scale=T, bias=per-row AP,
        accum_out): the sum of exp(T*(dist2 - K_i)) is a log-sum-exp max
        finished on the host with one ln(); the per-row offset
        K_i = a_i + c1 + c2*sqrt(a_i) + U keeps every row inside the f32
        exp window.  Rows outside the window (none on real data) are
        detected on the host and recomputed exactly.
  - Host: ln/sqrt/relu/mean in f64 (O(k) work) + the small exact slab.
"""

import sys

for _p in ("/opt/trn_rl_repo",):
    if _p not in sys.path:
        sys.path.insert(0, _p)

import numpy as np
import ml_dtypes

EPS = 1e-6
MARGIN = 0.2
B = 8192
D = 128
NCORES = 8
CW = 1024            # column chunk = 2 PSUM banks
NCH = B // CW

# log-sum-exp calibration (fit to randn(B,128) stats; host detector +
# exact-row fallback covers anything outside the window)
T = 1.0
BIASP = lambda sa: -(131.26 + 22.0 + 10.20 * sa) * T   # exp arg = T*v + bias

_programs = {}
LAST_RESULTS = None   # BassKernelResults of the most recent run (for profiling)


def _chunk_kinds(k):
    """Interleaved chunk layout: cp pos chunks, (NCH-cp) neg chunks."""
    cp = min(NCH - 1, max(1, k // CW))
    kinds = []
    np_, nn = cp, NCH - cp
    while np_ or nn:
        if np_:
            kinds.append("p")
            np_ -= 1
        if nn:
            kinds.append("n")
            nn -= 1
    return cp, kinds


def _build_program(cp, kp):
    import concourse.bacc as bacc
    import concourse.tile as tile
    from concourse import mybir

    f32 = mybir.dt.float32
    bf16 = mybir.dt.bfloat16
    AOT = mybir.AluOpType
    AFT = mybir.ActivationFunctionType

    SH = kp // NCORES
    NIT = SH // 128
    _, kinds = _chunk_kinds(cp * CW)
    nslot = NCH

    nc = bacc.Bacc(None)
    e1t = nc.declare_dram_parameter("e1t", [D, SH], bf16, isOutput=False)
    e2t = nc.declare_dram_parameter("e2t", [D, B], bf16, isOutput=False)
    # combined tail operands: [16, SH] weights then [16, B] rhs
    tcmb = nc.declare_dram_parameter("tcmb", [16, SH + B], bf16, isOutput=False)
    biasp = nc.declare_dram_parameter("biasp", [128, NIT], f32, isOutput=False)
    outp = nc.declare_dram_parameter("out", [128, NIT * nslot], f32, isOutput=True)

    with tile.TileContext(nc) as tc:
        with (
            tc.tile_pool(name="const", bufs=1) as cpool,
            tc.tile_pool(name="e2p", bufs=NCH) as e2pool,
            tc.tile_pool(name="ps", bufs=1, space="PSUM") as pspool,
        ):
            # sync HWDGE queue: e1, then the e2 chunks in use order
            e1sb = cpool.tile([D, SH], bf16, tag="e1sb")
            nc.sync.dma_start(e1sb[:], e1t[:])
            e2sb = []
            for g in range(NCH):
                e2c = e2pool.tile([D, CW], bf16, tag=f"e2c{g}")
                nc.sync.dma_start(e2c[:], e2t[:, g * CW:(g + 1) * CW])
                e2sb.append(e2c)

            # scalar HWDGE queue: bias + combined tail strips
            bpsb = cpool.tile([128, NIT], f32, tag="bpsb")
            nc.scalar.dma_start(bpsb[:], biasp[:])
            tcsb = cpool.tile([128, SH + B], bf16, tag="tcsb")
            for s in range(4):
                nc.scalar.dma_start(
                    tcsb[32 * s:32 * s + 4, :], tcmb[4 * s:4 * s + 4, :]
                )

            outsb = cpool.tile([128, NIT * nslot], f32, tag="outsb")
            scr = cpool.tile([128, CW], f32, tag="scr")

            # kick the Exp table load early so it overlaps input DMA
            nc.scalar.activation(scr[:, 0:1], bpsb[:, 0:1], AFT.Exp)

            ps = pspool.tile([128, 4096], f32, tag="ps")

            slot = 0
            for it in range(NIT):
                icols = slice(it * 128, (it + 1) * 128)
                w = e1sb[0:126, icols]
                for pi in range(NCH // 2):
                    gA = 2 * pi
                    roff = (pi % 2) * 2048      # PSUM half for this pair
                    # K=126 mains (embedding dims 0..125)
                    for gi in range(2):
                        for s in range(2):
                            c0 = roff + gi * CW + s * 512
                            nc.tensor.matmul(
                                ps[:, c0:c0 + 512],
                                w,
                                e2sb[gA + gi][0:126, s * 512:(s + 1) * 512],
                                start=True,
                                stop=False,
                            )
                    # K=4 tails (dims 126,127 + bias hi/lo), 4-way row-packed
                    # so the four 512-wide sub-tiles run concurrently
                    for si in range(4):
                        c0 = roff + si * 512
                        j0 = SH + gA * CW + si * 512
                        nc.tensor.matmul(
                            ps[:, c0:c0 + 512],
                            tcsb[32 * si:32 * si + 4, icols],
                            tcsb[32 * si:32 * si + 4, j0:j0 + 512],
                            start=False,
                            stop=True,
                            tile_position=(32 * si, 0),
                        )
                    # concurrent consumers: one engine per chunk of the pair
                    for gi in range(2):
                        dst = outsb[:, slot:slot + 1]
                        slot += 1
                        l = roff + gi * CW
                        if kinds[gA + gi] == "n":
                            nc.vector.tensor_reduce(
                                dst, ps[:, l:l + CW],
                                axis=mybir.AxisListType.X, op=AOT.min,
                            )
                        else:
                            nc.scalar.activation(
                                scr[:, 0:CW], ps[:, l:l + CW], AFT.Exp,
                                bias=bpsb[:, it:it + 1],
                                scale=T,
                                accum_out=dst,
                            )
                # drain this i-tile's results while compute continues
                nc.sync.dma_start(
                    outp[:, it * nslot:(it + 1) * nslot],
                    outsb[:, it * nslot:(it + 1) * nslot],
                )
    nc.compile()
    return nc


def _host_prep(emb1, emb2, target, kp):
    tpos = target == 1
    k = int(tpos.sum())
    pos_idx = np.nonzero(tpos)[0]
    perm = np.concatenate([pos_idx, np.nonzero(~tpos)[0]])
    e2s = emb2[perm]
    e2d = e2s.astype(np.float64)
    b = (e2d * e2d).sum(1) - (2.0 * EPS) * e2d.sum(1)

    # interleaved device column order (indices into the sorted order)
    cp, kinds = _chunk_kinds(k)
    colperm = np.empty(B, dtype=np.int64)
    ppos = 0
    pneg = k
    off = 0
    for kind in kinds:
        if kind == "p":
            colperm[off:off + CW] = np.arange(ppos, ppos + CW)
            ppos += CW
        else:
            n = min(CW, B - pneg)
            colperm[off:off + n] = np.arange(pneg, pneg + n)
            if n < CW:  # pad with duplicates of the first neg column
                colperm[off + n:off + CW] = k
            pneg += n
        off += CW

    e1dev = emb1[pos_idx[:kp]]
    e1d = e1dev.astype(np.float64)
    a = (e1d * e1d).sum(1) + (2.0 * EPS) * e1d.sum(1) + D * EPS * EPS
    e1tb = np.ascontiguousarray((-2.0 * e1dev).T.astype(ml_dtypes.bfloat16))
    e2dev = e2s[colperm]
    bdev = b[colperm]
    e2tb = np.ascontiguousarray(e2dev.T.astype(ml_dtypes.bfloat16))
    bhi = bdev.astype(np.float32).astype(ml_dtypes.bfloat16)
    blo = (bdev.astype(np.float32) - bhi.astype(np.float32)).astype(ml_dtypes.bfloat16)
    # K=4 tail operands, combined [16, SH+B]: cols [0,SH) weights (e1 dims
    # 126/127 + ones), cols [SH,SH+B) rhs (e2 dims 126/127 + bias hi/lo).
    # On device row 4s+r lands at partition 32s+r so the four 512-wide
    # sub-tiles of a pair can row-pack on the PE array.
    SH = kp // NCORES  # per-core; tcmb built per core in kernel()
    tailw = np.zeros((16, kp), dtype=ml_dtypes.bfloat16)
    trhs = np.zeros((16, B), dtype=ml_dtypes.bfloat16)
    one = np.ones(B, dtype=ml_dtypes.bfloat16)
    for s in range(4):
        tailw[4 * s + 0] = e1tb[126]
        tailw[4 * s + 1] = e1tb[127]
        tailw[4 * s + 2] = one[:kp]
        tailw[4 * s + 3] = one[:kp]
        trhs[4 * s + 0] = e2tb[126]
        trhs[4 * s + 1] = e2tb[127]
        trhs[4 * s + 2] = bhi
        trhs[4 * s + 3] = blo
    bp = BIASP(np.sqrt(a)).astype(np.float32)   # exp arg = T*v + bp
    return k, cp, kinds, a, b, e2d, pos_idx, e1tb, e2tb, tailw, trhs, bp


def _exact_rows(e1rows, e2d, b, k):
    """Exact f64 pos_max2/neg_min2 for a handful of anchor rows."""
    e1d = e1rows.astype(np.float64)
    av = (e1d * e1d).sum(1) + (2.0 * EPS) * e1d.sum(1) + D * EPS * EPS
    d2 = av[:, None] + b[None, :] - 2.0 * (e1d @ e2d.T)
    return d2[:, :k].max(1), d2[:, k:].min(1)


def _numpy_fallback(emb1, emb2, target):
    e1 = emb1.astype(np.float64)
    e2 = emb2.astype(np.float64)
    sq = (
        (e1 * e1).sum(1)[:, None]
        + (e2 * e2).sum(1)[None, :]
        - 2.0 * (e1 @ e2.T)
        + 2.0 * EPS * (e1.sum(1)[:, None] - e2.sum(1)[None, :])
        + D * EPS * EPS
    )
    dist = np.sqrt(np.clip(sq, 0.0, None))
    pos = target == 1
    neg = target == 0
    pos_max = np.where(pos[None, :], dist, -np.inf).max(1)
    neg_min = np.where(neg[None, :], dist, np.inf).min(1)
    per = np.maximum(pos_max - neg_min + MARGIN, 0.0)
    w = pos.astype(np.float64)
    return np.float32((per * w).sum() / w.sum())


def kernel(emb1, emb2, target):
    global LAST_RESULTS
    emb1 = np.asarray(emb1, dtype=np.float32)
    emb2 = np.asarray(emb2, dtype=np.float32)
    target = np.asarray(target)
    assert emb1.shape == (B, D) and emb2.shape == (B, D)

    k = int((target == 1).sum())
    kp = (k // (NCORES * 128)) * (NCORES * 128)
    if kp == 0 or k >= B - 1:
        return _numpy_fallback(emb1, emb2, target)

    k, cp, kinds, a, b, e2d, pos_idx, e1tb, e2tb, tailw, trhs, bp = _host_prep(
        emb1, emb2, target, kp
    )

    nc = _programs.get((cp, kp))
    if nc is None:
        nc = _build_program(cp, kp)
        _programs[(cp, kp)] = nc

    from concourse.bass_utils import run_bass_kernel_spmd

    SH = kp // NCORES
    NIT = SH // 128
    in_maps = [
        {
            "e1t": np.ascontiguousarray(e1tb[:, c * SH:(c + 1) * SH]),
            "e2t": e2tb,
            "tcmb": np.ascontiguousarray(
                np.concatenate([tailw[:, c * SH:(c + 1) * SH], trhs], axis=1)
            ),
            "biasp": np.ascontiguousarray(
                bp[c * SH:(c + 1) * SH].reshape(NIT, 128).T
            ),
        }
        for c in range(NCORES)
    ]
    res = run_bass_kernel_spmd(nc, in_maps, core_ids=list(range(NCORES)))
    LAST_RESULTS = res

    # ---- host reconstruction (all f64) ----
    nslot = NCH
    slot_ap = [i for i, kind in enumerate(kinds) if kind == "p"]
    slot_dv = [i for i, kind in enumerate(kinds) if kind == "n"]

    Sp = np.zeros(kp)
    vmin = np.full(kp, np.inf)
    for c in range(NCORES):
        out = np.asarray(res.results[c]["out"]).astype(np.float64)  # [128, NIT*nslot]
        for it in range(NIT):
            rows = slice(c * SH + it * 128, c * SH + (it + 1) * 128)
            blk = out[:, it * nslot:(it + 1) * nslot]
            Sp[rows] += blk[:, slot_ap].sum(1)
            vmin[rows] = np.minimum(vmin[rows], blk[:, slot_dv].min(1))

    bp64 = bp.astype(np.float64)
    with np.errstate(divide="ignore", invalid="ignore"):
        pm2 = a + (np.log(Sp) - bp64) / T
    nm2 = a + vmin

    # exact host slab: pos cols [cp*CW, k)
    if k > cp * CW:
        e1d = emb1[pos_idx[:kp]].astype(np.float64)
        d2s = (
            a[:, None]
            + b[None, cp * CW:k]
            - 2.0 * (e1d @ e2d[cp * CW:k].T)
        )
        pm2 = np.maximum(pm2, d2s.max(1))

    # detector: rows where the LSE left the reliable window -> exact redo
    bad = (~np.isfinite(Sp)) | (Sp <= 0) | (np.log(np.maximum(Sp, 1e-300)) < -60.0)
    bad |= ~np.isfinite(pm2) | ~np.isfinite(nm2)
    if bad.any():
        idx = np.nonzero(bad)[0]
        pmx, nmx = _exact_rows(emb1[pos_idx[idx]], e2d, b, k)
        pm2[idx] = pmx
        nm2[idx] = nmx

    per = np.maximum(
        np.sqrt(np.clip(pm2, 0.0, None)) - np.sqrt(np.clip(nm2, 0.0, None)) + MARGIN,
        0.0,
    )
    total = per.sum()

    if k > kp:  # leftover pos anchors, exact on host
        pmx, nmx = _exact_rows(emb1[pos_idx[kp:k]], e2d, b, k)
        total += np.maximum(
            np.sqrt(np.clip(pmx, 0.0, None)) - np.sqrt(np.clip(nmx, 0.0, None)) + MARGIN,
            0.0,
        ).sum()

    return np.float32(total / k)


# revision 18
# speedup vs baseline: 1.2131x; 1.0741x over previous
"""BatchHardTripletLoss kernel for 8 Trainium2 NeuronCores.

Math (matches the jax reference):
  dist2[i,j] = |e1_i|^2 + |e2_j|^2 - 2 e1.e2 + 2*eps*(s1_i - s2_j) + D*eps^2
             = a[i] + v[i,j],   v[i,j] = b[j] - 2*G[i,j]
  pos_max[i] = sqrt(max_{j in pos} dist2), neg_min[i] = sqrt(min_{j in neg})
  loss = mean over pos anchors of relu(pos_max - neg_min + margin)

Only rows with target[i]==1 contribute, so the device computes pos anchors
only (~k/8 rows per core).  The k % 1024 leftover anchors and the pos
columns past the last full 1024-chunk are done exactly on the host (tiny
numpy job).

Device strategy (data parallel over pos-anchor rows, 8 cores):
  - Host INTERLEAVES pos/neg candidate columns in 1024-col chunks
    (P N P N ...), so every 2048-col chunk pair feeds BOTH consumer
    engines at once (chunk boundaries stay PSUM-bank aligned -- a
    mid-bank split makes ScalarE and VectorE contend on one bank).
    emb2 goes up transposed [D, cols] bf16; emb1 is scaled by -2; the
    bias b[j] enters PSUM via K=4 row-packed tail matmuls (dims 126/127
    + bf16 hi/lo split of b) next to the K=126 mains.  Neg chunks are
    padded to 1024 with duplicate neg columns.
  - PSUM is one [128, 4096] f32 tile = ring of two 2048-col halves.  Per
    (i-tile of 128 anchors, chunk pair) TensorE fills one half; pairs
    alternate halves; the Tile framework's range-level dependencies give
    region-granular pipelining.  In steady state all three engines are
    >90% busy and the PE never idles, which keeps the HAM activity
    clock-gate at 2.4 GHz (the first couple of cold pairs warm it up).
    Input DMAs are split across the Sync (e1, e2 chunks in use order)
    and Scalar (bias + combined tail strips) HWDGE queues so the first
    pair can start ~1 DMA-issue after the framework preamble.
  - Consumers run concurrently per pair:
      neg chunk -> VectorE exact tensor_reduce(min) of v.
      pos chunk -> ScalarE activation(Exp, scale=T, bias=per-row AP,
        accum_out): the sum of exp(T*(dist2 - K_i)) is a log-sum-exp max
        finished on the host with one ln(); the per-row offset
        K_i = a_i + c1 + c2*sqrt(a_i) + U keeps every row inside the f32
        exp window.  Rows outside the window (none on real data) are
        detected on the host and recomputed exactly.
  - Host: ln/sqrt/relu/mean in f64 (O(k) work) + the small exact slab.
"""

import sys

for _p in ("/opt/trn_rl_repo",):
    if _p not in sys.path:
        sys.path.insert(0, _p)

import numpy as np
import ml_dtypes

EPS = 1e-6
MARGIN = 0.2
B = 8192
D = 128
NCORES = 8
CW = 1024            # column chunk = 2 PSUM banks
NCH = B // CW

# log-sum-exp calibration (fit to randn(B,128) stats; host detector +
# exact-row fallback covers anything outside the window)
T = 1.0
BIASP = lambda sa: -(131.26 + 22.0 + 10.20 * sa) * T   # exp arg = T*v + bias

_programs = {}
LAST_RESULTS = None   # BassKernelResults of the most recent run (for profiling)


def _chunk_kinds(k):
    """Interleaved chunk layout: cp pos chunks, (NCH-cp) neg chunks."""
    cp = min(NCH - 1, max(1, k // CW))
    kinds = []
    np_, nn = cp, NCH - cp
    while np_ or nn:
        if np_:
            kinds.append("p")
            np_ -= 1
        if nn:
            kinds.append("n")
            nn -= 1
    return cp, kinds


def _build_program(cp, kp):
    import concourse.bacc as bacc
    import concourse.tile as tile
    from concourse import mybir

    f32 = mybir.dt.float32
    bf16 = mybir.dt.bfloat16
    AOT = mybir.AluOpType
    AFT = mybir.ActivationFunctionType

    SH = kp // NCORES
    NIT = SH // 128
    _, kinds = _chunk_kinds(cp * CW)
    nslot = NCH

    nc = bacc.Bacc(None)
    e1t = nc.declare_dram_parameter("e1t", [D, SH], bf16, isOutput=False)
    e2t = nc.declare_dram_parameter("e2t", [D, B], bf16, isOutput=False)
    tailw = nc.declare_dram_parameter("tailw", [16, SH], bf16, isOutput=False)
    trhs = nc.declare_dram_parameter("trhs", [16, B], bf16, isOutput=False)
    biasp = nc.declare_dram_parameter("biasp", [128, NIT], f32, isOutput=False)
    outp = nc.declare_dram_parameter("out", [128, NIT * nslot], f32, isOutput=True)

    with tile.TileContext(nc) as tc:
        with (
            tc.tile_pool(name="const", bufs=1) as cpool,
            tc.tile_pool(name="e2p", bufs=NCH) as e2pool,
            tc.tile_pool(name="ps", bufs=1, space="PSUM") as pspool,
        ):
            # sync HWDGE queue: e1, then the e2 chunks in use order
            e1sb = cpool.tile([D, SH], bf16, tag="e1sb")
            nc.sync.dma_start(e1sb[:], e1t[:])
            e2sb = []
            for g in range(NCH):
                e2c = e2pool.tile([D, CW], bf16, tag=f"e2c{g}")
                nc.sync.dma_start(e2c[:], e2t[:, g * CW:(g + 1) * CW])
                e2sb.append(e2c)

            # scalar HWDGE queue (idle until the first ACTIVATE): bias and
            # the tail strips; both matmul operands must share the 32s
            # partition base, so weights and rhs are replicated per strip
            bpsb = cpool.tile([128, NIT], f32, tag="bpsb")
            nc.scalar.dma_start(bpsb[:], biasp[:])
            trsb = cpool.tile([128, B], bf16, tag="trsb")
            for s in range(4):
                nc.scalar.dma_start(trsb[32 * s:32 * s + 4, :], trhs[4 * s:4 * s + 4, :])
            twsb = cpool.tile([128, SH], bf16, tag="twsb")
            for s in range(4):
                nc.scalar.dma_start(
                    twsb[32 * s:32 * s + 4, :], tailw[4 * s:4 * s + 4, :]
                )

            outsb = cpool.tile([128, NIT * nslot], f32, tag="outsb")
            scr = cpool.tile([128, CW], f32, tag="scr")

            # kick the Exp table load early so it overlaps input DMA
            nc.scalar.activation(scr[:, 0:1], bpsb[:, 0:1], AFT.Exp)

            ps = pspool.tile([128, 4096], f32, tag="ps")

            # warm the PE activity window with throwaway matmuls (one clean
            # ~3.4us burst flips the HAM clock gate to 2.4 GHz; micro-gaps in
            # steady state don't drop it back, but a cold start never flips)
            for _ in range(8):
                nc.tensor.matmul(
                    ps[:, 0:512], e1sb[0:126, 0:128], e1sb[0:126, 0:512],
                    start=True, stop=True,
                )

            slot = 0
            for it in range(NIT):
                icols = slice(it * 128, (it + 1) * 128)
                w = e1sb[0:126, icols]
                for pi in range(NCH // 2):
                    gA = 2 * pi
                    roff = (pi % 2) * 2048      # PSUM half for this pair
                    # K=126 mains (embedding dims 0..125)
                    for gi in range(2):
                        for s in range(2):
                            c0 = roff + gi * CW + s * 512
                            nc.tensor.matmul(
                                ps[:, c0:c0 + 512],
                                w,
                                e2sb[gA + gi][0:126, s * 512:(s + 1) * 512],
                                start=True,
                                stop=False,
                            )
                    # K=4 tails (dims 126,127 + bias hi/lo), 4-way row-packed
                    # so the four 512-wide sub-tiles run concurrently
                    for si in range(4):
                        c0 = roff + si * 512
                        j0 = gA * CW + si * 512
                        nc.tensor.matmul(
                            ps[:, c0:c0 + 512],
                            twsb[32 * si:32 * si + 4, icols],
                            trsb[32 * si:32 * si + 4, j0:j0 + 512],
                            start=False,
                            stop=True,
                            tile_position=(32 * si, 0),
                        )
                    # concurrent consumers: one engine per chunk of the pair
                    for gi in range(2):
                        dst = outsb[:, slot:slot + 1]
                        slot += 1
                        l = roff + gi * CW
                        if kinds[gA + gi] == "n":
                            nc.vector.tensor_reduce(
                                dst, ps[:, l:l + CW],
                                axis=mybir.AxisListType.X, op=AOT.min,
                            )
                        else:
                            nc.scalar.activation(
                                scr[:, 0:CW], ps[:, l:l + CW], AFT.Exp,
                                bias=bpsb[:, it:it + 1],
                                scale=T,
                                accum_out=dst,
                            )
                # drain this i-tile's results while compute continues
                nc.sync.dma_start(
                    outp[:, it * nslot:(it + 1) * nslot],
                    outsb[:, it * nslot:(it + 1) * nslot],
                )
    nc.compile()
    return nc


def _host_prep(emb1, emb2, target, kp):
    tpos = target == 1
    k = int(tpos.sum())
    pos_idx = np.nonzero(tpos)[0]
    perm = np.concatenate([pos_idx, np.nonzero(~tpos)[0]])
    e2s = emb2[perm]
    e2d = e2s.astype(np.float64)
    b = (e2d * e2d).sum(1) - (2.0 * EPS) * e2d.sum(1)

    # interleaved device column order (indices into the sorted order)
    cp, kinds = _chunk_kinds(k)
    colperm = np.empty(B, dtype=np.int64)
    ppos = 0
    pneg = k
    off = 0
    for kind in kinds:
        if kind == "p":
            colperm[off:off + CW] = np.arange(ppos, ppos + CW)
            ppos += CW
        else:
            n = min(CW, B - pneg)
            colperm[off:off + n] = np.arange(pneg, pneg + n)
            if n < CW:  # pad with duplicates of the first neg column
                colperm[off + n:off + CW] = k
            pneg += n
        off += CW

    e1dev = emb1[pos_idx[:kp]]
    e1d = e1dev.astype(np.float64)
    a = (e1d * e1d).sum(1) + (2.0 * EPS) * e1d.sum(1) + D * EPS * EPS
    e1tb = np.ascontiguousarray((-2.0 * e1dev).T.astype(ml_dtypes.bfloat16))
    e2dev = e2s[colperm]
    bdev = b[colperm]
    e2tb = np.ascontiguousarray(e2dev.T.astype(ml_dtypes.bfloat16))
    bhi = bdev.astype(np.float32).astype(ml_dtypes.bfloat16)
    blo = (bdev.astype(np.float32) - bhi.astype(np.float32)).astype(ml_dtypes.bfloat16)
    # K=4 tail operands (e1 dims 126/127 + ones | e2 dims 126/127 + bias
    # hi/lo), replicated per 32-partition strip for the four PE row tiles
    tailw = np.zeros((16, kp), dtype=ml_dtypes.bfloat16)
    trhs = np.zeros((16, B), dtype=ml_dtypes.bfloat16)
    one = np.ones(B, dtype=ml_dtypes.bfloat16)
    for s in range(4):
        tailw[4 * s + 0] = e1tb[126]
        tailw[4 * s + 1] = e1tb[127]
        tailw[4 * s + 2] = one[:kp]
        tailw[4 * s + 3] = one[:kp]
        trhs[4 * s + 0] = e2tb[126]
        trhs[4 * s + 1] = e2tb[127]
        trhs[4 * s + 2] = bhi
        trhs[4 * s + 3] = blo
    bp = BIASP(np.sqrt(a)).astype(np.float32)   # exp arg = T*v + bp
    return k, cp, kinds, a, b, e2d, pos_idx, e1tb, e2tb, tailw, trhs, bp


def _exact_rows(e1rows, e2d, b, k):
    """Exact f64 pos_max2/neg_min2 for a handful of anchor rows."""
    e1d = e1rows.astype(np.float64)
    av = (e1d * e1d).sum(1) + (2.0 * EPS) * e1d.sum(1) + D * EPS * EPS
    d2 = av[:, None] + b[None, :] - 2.0 * (e1d @ e2d.T)
    return d2[:, :k].max(1), d2[:, k:].min(1)


def _numpy_fallback(emb1, emb2, target):
    e1 = emb1.astype(np.float64)
    e2 = emb2.astype(np.float64)
    sq = (
        (e1 * e1).sum(1)[:, None]
        + (e2 * e2).sum(1)[None, :]
        - 2.0 * (e1 @ e2.T)
        + 2.0 * EPS * (e1.sum(1)[:, None] - e2.sum(1)[None, :])
        + D * EPS * EPS
    )
    dist = np.sqrt(np.clip(sq, 0.0, None))
    pos = target == 1
    neg = target == 0
    pos_max = np.where(pos[None, :], dist, -np.inf).max(1)
    neg_min = np.where(neg[None, :], dist, np.inf).min(1)
    per = np.maximum(pos_max - neg_min + MARGIN, 0.0)
    w = pos.astype(np.float64)
    return np.float32((per * w).sum() / w.sum())


def kernel(emb1, emb2, target):
    global LAST_RESULTS
    emb1 = np.asarray(emb1, dtype=np.float32)
    emb2 = np.asarray(emb2, dtype=np.float32)
    target = np.asarray(target)
    assert emb1.shape == (B, D) and emb2.shape == (B, D)

    k = int((target == 1).sum())
    kp = (k // (NCORES * 128)) * (NCORES * 128)
    if kp == 0 or k >= B - 1:
        return _numpy_fallback(emb1, emb2, target)

    k, cp, kinds, a, b, e2d, pos_idx, e1tb, e2tb, tailw, trhs, bp = _host_prep(
        emb1, emb2, target, kp
    )

    nc = _programs.get((cp, kp))
    if nc is None:
        nc = _build_program(cp, kp)
        _programs[(cp, kp)] = nc

    from concourse.bass_utils import run_bass_kernel_spmd

    SH = kp // NCORES
    NIT = SH // 128
    in_maps = [
        {
            "e1t": np.ascontiguousarray(e1tb[:, c * SH:(c + 1) * SH]),
            "e2t": e2tb,
            "tailw": np.ascontiguousarray(tailw[:, c * SH:(c + 1) * SH]),
            "trhs": trhs,
            "biasp": np.ascontiguousarray(
                bp[c * SH:(c + 1) * SH].reshape(NIT, 128).T
            ),
        }
        for c in range(NCORES)
    ]
    res = run_bass_kernel_spmd(nc, in_maps, core_ids=list(range(NCORES)))
    LAST_RESULTS = res

    # ---- host reconstruction (all f64) ----
    nslot = NCH
    slot_ap = [i for i, kind in enumerate(kinds) if kind == "p"]
    slot_dv = [i for i, kind in enumerate(kinds) if kind == "n"]

    Sp = np.zeros(kp)
    vmin = np.full(kp, np.inf)
    for c in range(NCORES):
        out = np.asarray(res.results[c]["out"]).astype(np.float64)  # [128, NIT*nslot]
        for it in range(NIT):
            rows = slice(c * SH + it * 128, c * SH + (it + 1) * 128)
            blk = out[:, it * nslot:(it + 1) * nslot]
            Sp[rows] += blk[:, slot_ap].sum(1)
            vmin[rows] = np.minimum(vmin[rows], blk[:, slot_dv].min(1))

    bp64 = bp.astype(np.float64)
    with np.errstate(divide="ignore", invalid="ignore"):
        pm2 = a + (np.log(Sp) - bp64) / T
    nm2 = a + vmin

    # exact host slab: pos cols [cp*CW, k)
    if k > cp * CW:
        e1d = emb1[pos_idx[:kp]].astype(np.float64)
        d2s = (
            a[:, None]
            + b[None, cp * CW:k]
            - 2.0 * (e1d @ e2d[cp * CW:k].T)
        )
        pm2 = np.maximum(pm2, d2s.max(1))

    # detector: rows where the LSE left the reliable window -> exact redo
    bad = (~np.isfinite(Sp)) | (Sp <= 0) | (np.log(np.maximum(Sp, 1e-300)) < -60.0)
    bad |= ~np.isfinite(pm2) | ~np.isfinite(nm2)
    if bad.any():
        idx = np.nonzero(bad)[0]
        pmx, nmx = _exact_rows(emb1[pos_idx[idx]], e2d, b, k)
        pm2[idx] = pmx
        nm2[idx] = nmx

    per = np.maximum(
        np.sqrt(np.clip(pm2, 0.0, None)) - np.sqrt(np.clip(nm2, 0.0, None)) + MARGIN,
        0.0,
    )
    total = per.sum()

    if k > kp:  # leftover pos anchors, exact on host
        pmx, nmx = _exact_rows(emb1[pos_idx[kp:k]], e2d, b, k)
        total += np.maximum(
            np.sqrt(np.clip(pmx, 0.0, None)) - np.sqrt(np.clip(nmx, 0.0, None)) + MARGIN,
            0.0,
        ).sum()

    return np.float32(total / k)


# revision 23
# speedup vs baseline: 1.5476x; 1.2757x over previous
"""BatchHardTripletLoss kernel for 8 Trainium2 NeuronCores.

Math (matches the jax reference):
  dist2[i,j] = |e1_i|^2 + |e2_j|^2 - 2 e1.e2 + 2*eps*(s1_i - s2_j) + D*eps^2
             = a[i] + v[i,j],   v[i,j] = b[j] - 2*G[i,j]
  pos_max[i] = sqrt(max_{j in pos} dist2), neg_min[i] = sqrt(min_{j in neg})
  loss = mean over pos anchors of relu(pos_max - neg_min + margin)

Only rows with target[i]==1 contribute, so the device computes pos anchors
only (~k/8 rows per core).  The k % 1024 leftover anchors and the pos
columns past the last full 1024-chunk are done exactly on the host (tiny
numpy job).

Device strategy (data parallel over pos-anchor rows, 8 cores):
  - Host INTERLEAVES pos/neg candidate columns in 1024-col chunks
    (P N P N ...), so every 2048-col chunk pair feeds BOTH consumer
    engines at once (chunk boundaries stay PSUM-bank aligned -- a
    mid-bank split makes ScalarE and VectorE contend on one bank).
    emb2 goes up transposed [D, cols] bf16; emb1 is scaled by -2; the
    bias b[j] enters PSUM via K=4 row-packed tail matmuls (dims 126/127
    + bf16 hi/lo split of b) next to the K=126 mains.  Neg chunks are
    padded to 1024 with duplicate neg columns.
  - PSUM is one [128, 4096] f32 tile = ring of two 2048-col halves.  Per
    (i-tile of 128 anchors, chunk pair) TensorE fills one half; pairs
    alternate halves; the Tile framework's range-level dependencies give
    region-granular pipelining.  In steady state all three engines are
    >90% busy and the PE never idles, which keeps the HAM activity
    clock-gate at 2.4 GHz (the first couple of cold pairs warm it up).
    Input DMAs are split across the Sync (e1, e2 chunks in use order)
    and Scalar (bias + combined tail strips) HWDGE queues so the first
    pair can start ~1 DMA-issue after the framework preamble.
  - Consumers run concurrently per pair:
      neg chunk -> VectorE exact tensor_reduce(min) of v.
      pos chunk -> ScalarE activation(Exp, scale=T, bias=per-row AP,
        accum_out): the sum of exp(T*(dist2 - K_i)) is a log-sum-exp max
        finished on the host with one ln(); the per-row offset
        K_i = a_i + c1 + c2*sqrt(a_i) + U keeps every row inside the f32
        exp window.  Rows outside the window (none on real data) are
        detected on the host and recomputed exactly.
  - Host: ln/sqrt/relu/mean in f64 (O(k) work) + the small exact slab.
"""

import sys

for _p in ("/opt/trn_rl_repo",):
    if _p not in sys.path:
        sys.path.insert(0, _p)

import numpy as np
import ml_dtypes

EPS = 1e-6
MARGIN = 0.2
B = 8192
D = 128
NCORES = 8
CW = 1024            # column chunk = 2 PSUM banks
NCH = B // CW

# log-sum-exp calibration (fit to randn(B,128) stats; host detector +
# exact-row fallback covers anything outside the window)
T = 1.0
BIASP = lambda sa: -(131.26 + 22.0 + 10.20 * sa) * T   # exp arg = T*v + bias

_programs = {}
LAST_RESULTS = None   # BassKernelResults of the most recent run (for profiling)


def _chunk_kinds(k):
    """Interleaved chunk layout: cp pos chunks, (NCH-cp) neg chunks."""
    cp = min(NCH - 1, max(1, k // CW))
    kinds = []
    np_, nn = cp, NCH - cp
    while np_ or nn:
        if np_:
            kinds.append("p")
            np_ -= 1
        if nn:
            kinds.append("n")
            nn -= 1
    return cp, kinds


def _build_program(cp, kp):
    import concourse.bacc as bacc
    import concourse.tile as tile
    from concourse import mybir

    f32 = mybir.dt.float32
    bf16 = mybir.dt.bfloat16
    AOT = mybir.AluOpType
    AFT = mybir.ActivationFunctionType

    SH = kp // NCORES
    NIT = SH // 128
    _, kinds = _chunk_kinds(cp * CW)
    nslot = NCH

    nc = bacc.Bacc(None)
    e1t = nc.declare_dram_parameter("e1t", [D, SH], bf16, isOutput=False)
    e2t = nc.declare_dram_parameter("e2t", [D, B], bf16, isOutput=False)
    trhs = nc.declare_dram_parameter("trhs", [8, B], bf16, isOutput=False)
    biasp = nc.declare_dram_parameter("biasp", [128, NIT], f32, isOutput=False)
    outp = nc.declare_dram_parameter("out", [128, NIT * nslot], f32, isOutput=True)

    with tile.TileContext(nc) as tc:
        with (
            tc.tile_pool(name="const", bufs=1) as cpool,
            tc.tile_pool(name="e2p", bufs=NCH) as e2pool,
            tc.tile_pool(name="ps", bufs=1, space="PSUM") as pspool,
        ):
            # sync HWDGE queue: e1, bias-tail strips and e2 chunks in use order
            e1sb = cpool.tile([D, SH], bf16, tag="e1sb")
            nc.sync.dma_start(e1sb[:], e1t[:])
            trsb = cpool.tile([128, B], bf16, tag="trsb")
            e2sb = []
            def load_chunk(g):
                e2c = e2pool.tile([D, CW], bf16, tag=f"e2c{g}")
                nc.sync.dma_start(e2c[:], e2t[:, g * CW:(g + 1) * CW])
                e2sb.append(e2c)
            for s in range(2):
                nc.sync.dma_start(trsb[32 * s:32 * s + 2, :], trhs[2 * s:2 * s + 2, :])
            load_chunk(0)
            load_chunk(1)
            for s in range(2, 4):
                nc.sync.dma_start(trsb[32 * s:32 * s + 2, :], trhs[2 * s:2 * s + 2, :])
            for g in range(2, NCH):
                load_chunk(g)

            # scalar HWDGE queue: bias only
            bpsb = cpool.tile([128, NIT], f32, tag="bpsb")
            nc.scalar.dma_start(bpsb[:], biasp[:])

            # tail weights are all-ones: build on device, no DMA
            twsb = cpool.tile([128, SH], bf16, tag="twsb")
            nc.gpsimd.memset(twsb[:], 1.0)

            outsb = cpool.tile([128, NIT * nslot], f32, tag="outsb")
            scr = cpool.tile([128, CW], f32, tag="scr")

            # kick the Exp table load early so it overlaps input DMA
            nc.scalar.activation(scr[:, 0:1], bpsb[:, 0:1], AFT.Exp)

            ps = pspool.tile([128, 4096], f32, tag="ps")

            # warm the PE activity window with throwaway matmuls (one clean
            # ~3.4us burst flips the HAM clock gate to 2.4 GHz; micro-gaps in
            # steady state don't drop it back, but a cold start never flips)
            for _ in range(8):
                nc.tensor.matmul(
                    ps[:, 0:512], e1sb[0:128, 0:128], e1sb[0:128, 0:512],
                    start=True, stop=True,
                )

            slot = 0
            for it in range(NIT):
                icols = slice(it * 128, (it + 1) * 128)
                w = e1sb[0:128, icols]
                for pi in range(NCH // 2):
                    gA = 2 * pi
                    roff = (pi % 2) * 2048      # PSUM half for this pair
                    # K=128 mains (all embedding dims)
                    for gi in range(2):
                        for s in range(2):
                            c0 = roff + gi * CW + s * 512
                            nc.tensor.matmul(
                                ps[:, c0:c0 + 512],
                                w,
                                e2sb[gA + gi][0:128, s * 512:(s + 1) * 512],
                                start=True,
                                stop=False,
                            )
                    # K=2 bias tails (ones x bf16 hi/lo split of b), 4-way
                    # row-packed so the four 512-wide sub-tiles run
                    # concurrently
                    for si in range(4):
                        c0 = roff + si * 512
                        j0 = gA * CW + si * 512
                        nc.tensor.matmul(
                            ps[:, c0:c0 + 512],
                            twsb[32 * si:32 * si + 2, icols],
                            trsb[32 * si:32 * si + 2, j0:j0 + 512],
                            start=False,
                            stop=True,
                            tile_position=(32 * si, 0),
                        )
                    # concurrent consumers: one engine per chunk of the pair
                    for gi in range(2):
                        dst = outsb[:, slot:slot + 1]
                        slot += 1
                        l = roff + gi * CW
                        if kinds[gA + gi] == "n":
                            nc.vector.tensor_reduce(
                                dst, ps[:, l:l + CW],
                                axis=mybir.AxisListType.X, op=AOT.min,
                            )
                        else:
                            nc.scalar.activation(
                                scr[:, 0:CW], ps[:, l:l + CW], AFT.Exp,
                                bias=bpsb[:, it:it + 1],
                                scale=T,
                                accum_out=dst,
                            )
                # drain this i-tile's results while compute continues
                nc.sync.dma_start(
                    outp[:, it * nslot:(it + 1) * nslot],
                    outsb[:, it * nslot:(it + 1) * nslot],
                )
    nc.compile()
    return nc


def _host_prep(emb1, emb2, target, kp):
    tpos = target == 1
    k = int(tpos.sum())
    pos_idx = np.nonzero(tpos)[0]
    perm = np.concatenate([pos_idx, np.nonzero(~tpos)[0]])
    e2s = emb2[perm]
    e2d = e2s.astype(np.float64)
    b = (e2d * e2d).sum(1) - (2.0 * EPS) * e2d.sum(1)

    # interleaved device column order (indices into the sorted order)
    cp, kinds = _chunk_kinds(k)
    colperm = np.empty(B, dtype=np.int64)
    ppos = 0
    pneg = k
    off = 0
    for kind in kinds:
        if kind == "p":
            colperm[off:off + CW] = np.arange(ppos, ppos + CW)
            ppos += CW
        else:
            n = min(CW, B - pneg)
            colperm[off:off + n] = np.arange(pneg, pneg + n)
            if n < CW:  # pad with duplicates of the first neg column
                colperm[off + n:off + CW] = k
            pneg += n
        off += CW

    e1dev = emb1[pos_idx[:kp]]
    e1d = e1dev.astype(np.float64)
    a = (e1d * e1d).sum(1) + (2.0 * EPS) * e1d.sum(1) + D * EPS * EPS
    e1tb = np.ascontiguousarray((-2.0 * e1dev).T.astype(ml_dtypes.bfloat16))
    e2dev = e2s[colperm]
    bdev = b[colperm]
    e2tb = np.ascontiguousarray(e2dev.T.astype(ml_dtypes.bfloat16))
    bhi = bdev.astype(np.float32).astype(ml_dtypes.bfloat16)
    blo = (bdev.astype(np.float32) - bhi.astype(np.float32)).astype(ml_dtypes.bfloat16)
    # K=2 bias tail rhs (bf16 hi/lo split of b), replicated per
    # 32-partition strip for the four PE row tiles
    trhs = np.zeros((8, B), dtype=ml_dtypes.bfloat16)
    for s in range(4):
        trhs[2 * s + 0] = bhi
        trhs[2 * s + 1] = blo
    bp = BIASP(np.sqrt(a)).astype(np.float32)   # exp arg = T*v + bp
    return k, cp, kinds, a, b, e2d, pos_idx, e1tb, e2tb, trhs, bp


def _exact_rows(e1rows, e2d, b, k):
    """Exact f64 pos_max2/neg_min2 for a handful of anchor rows."""
    e1d = e1rows.astype(np.float64)
    av = (e1d * e1d).sum(1) + (2.0 * EPS) * e1d.sum(1) + D * EPS * EPS
    d2 = av[:, None] + b[None, :] - 2.0 * (e1d @ e2d.T)
    return d2[:, :k].max(1), d2[:, k:].min(1)


def _numpy_fallback(emb1, emb2, target):
    e1 = emb1.astype(np.float64)
    e2 = emb2.astype(np.float64)
    sq = (
        (e1 * e1).sum(1)[:, None]
        + (e2 * e2).sum(1)[None, :]
        - 2.0 * (e1 @ e2.T)
        + 2.0 * EPS * (e1.sum(1)[:, None] - e2.sum(1)[None, :])
        + D * EPS * EPS
    )
    dist = np.sqrt(np.clip(sq, 0.0, None))
    pos = target == 1
    neg = target == 0
    pos_max = np.where(pos[None, :], dist, -np.inf).max(1)
    neg_min = np.where(neg[None, :], dist, np.inf).min(1)
    per = np.maximum(pos_max - neg_min + MARGIN, 0.0)
    w = pos.astype(np.float64)
    return np.float32((per * w).sum() / w.sum())


def kernel(emb1, emb2, target):
    global LAST_RESULTS
    emb1 = np.asarray(emb1, dtype=np.float32)
    emb2 = np.asarray(emb2, dtype=np.float32)
    target = np.asarray(target)
    assert emb1.shape == (B, D) and emb2.shape == (B, D)

    k = int((target == 1).sum())
    kp = (k // (NCORES * 128)) * (NCORES * 128)
    if kp == 0 or k >= B - 1:
        return _numpy_fallback(emb1, emb2, target)

    k, cp, kinds, a, b, e2d, pos_idx, e1tb, e2tb, trhs, bp = _host_prep(
        emb1, emb2, target, kp
    )

    nc = _programs.get((cp, kp))
    if nc is None:
        nc = _build_program(cp, kp)
        _programs[(cp, kp)] = nc

    from concourse.bass_utils import run_bass_kernel_spmd

    SH = kp // NCORES
    NIT = SH // 128
    in_maps = [
        {
            "e1t": np.ascontiguousarray(e1tb[:, c * SH:(c + 1) * SH]),
            "e2t": e2tb,
            "trhs": trhs,
            "biasp": np.ascontiguousarray(
                bp[c * SH:(c + 1) * SH].reshape(NIT, 128).T
            ),
        }
        for c in range(NCORES)
    ]
    res = run_bass_kernel_spmd(nc, in_maps, core_ids=list(range(NCORES)))
    LAST_RESULTS = res

    # ---- host reconstruction (all f64) ----
    nslot = NCH
    slot_ap = [i for i, kind in enumerate(kinds) if kind == "p"]
    slot_dv = [i for i, kind in enumerate(kinds) if kind == "n"]

    Sp = np.zeros(kp)
    vmin = np.full(kp, np.inf)
    for c in range(NCORES):
        out = np.asarray(res.results[c]["out"]).astype(np.float64)  # [128, NIT*nslot]
        for it in range(NIT):
            rows = slice(c * SH + it * 128, c * SH + (it + 1) * 128)
            blk = out[:, it * nslot:(it + 1) * nslot]
            Sp[rows] += blk[:, slot_ap].sum(1)
            vmin[rows] = np.minimum(vmin[rows], blk[:, slot_dv].min(1))

    bp64 = bp.astype(np.float64)
    with np.errstate(divide="ignore", invalid="ignore"):
        pm2 = a + (np.log(Sp) - bp64) / T
    nm2 = a + vmin

    # exact host slab: pos cols [cp*CW, k)
    if k > cp * CW:
        e1d = emb1[pos_idx[:kp]].astype(np.float64)
        d2s = (
            a[:, None]
            + b[None, cp * CW:k]
            - 2.0 * (e1d @ e2d[cp * CW:k].T)
        )
        pm2 = np.maximum(pm2, d2s.max(1))

    # detector: rows where the LSE left the reliable window -> exact redo
    bad = (~np.isfinite(Sp)) | (Sp <= 0) | (np.log(np.maximum(Sp, 1e-300)) < -60.0)
    bad |= ~np.isfinite(pm2) | ~np.isfinite(nm2)
    if bad.any():
        idx = np.nonzero(bad)[0]
        pmx, nmx = _exact_rows(emb1[pos_idx[idx]], e2d, b, k)
        pm2[idx] = pmx
        nm2[idx] = nmx

    per = np.maximum(
        np.sqrt(np.clip(pm2, 0.0, None)) - np.sqrt(np.clip(nm2, 0.0, None)) + MARGIN,
        0.0,
    )
    total = per.sum()

    if k > kp:  # leftover pos anchors, exact on host
        pmx, nmx = _exact_rows(emb1[pos_idx[kp:k]], e2d, b, k)
        total += np.maximum(
            np.sqrt(np.clip(pmx, 0.0, None)) - np.sqrt(np.clip(nmx, 0.0, None)) + MARGIN,
            0.0,
        ).sum()

    return np.float32(total / k)
